# revision 8
# baseline (speedup 1.0000x reference)
"""DISCO S2 discrete-continuous convolution kernel for Trainium2 (8 cores).

Math (reference):
  xk[c,k,ho,wo] = sum_e [ker_e=k][row_e=ho] v_e * x[c, hi_e, (wi_e + 2*wo) % 720]
  out[o,ho,wo]  = sum_{c,k} w[o,c,k] * xk[c,k,ho,wo] + bias[o]

Fused single-stage formulation used here:
  out[o,ho,wo] = sum_{e: row_e=ho} sum_c (v_e * w[o,c,ker_e]) * x2[p_e, c, hi_e, s_e + wo]
where wi_e = 2*s_e + p_e and x2[p,c,hi,:] is the parity-p longitude
subsampling of x row hi, duplicated twice (720 wide) so the 360-wide
window starting at s_e never wraps.

Device mapping: contraction slots (e,c) go on the matmul K dim (128 per
chunk = 4 entries x 32 channels), M = 32 output channels, N = 360 output
longitudes.  lhsT [128,32] = v_e * W[:,c,k_e] precomputed on host.  rhs
[128,360] rows are gathered from x2 in DRAM by an indirect DMA whose
offset coefficient is patched to 1 for element-granular starts.
PSUM accumulates all chunks of one output latitude row.

Sharding: output latitude rows (181) are dealt to 8 cores x 23 slots,
rows ranked by entry count so every core's slot-j template chunk count
matches (single SPMD program, per-core data).
"""

import math
import sys

import numpy as np

if "/opt/trn_rl_repo" not in sys.path:
    sys.path.insert(0, "/opt/trn_rl_repo")

import concourse.bacc as bacc
import concourse.bass as bass
import concourse.mybir as mb
import concourse.tile as tile
from concourse import bass_utils
from concourse.bass import IndirectOffsetOnAxis

# ---------------- problem constants (hardcoded per contract) ----------------
C = 32          # input channels
O = 32          # output channels
KK = 9          # kernel size
HI, WI = 361, 720
HO, WO = 181, 360
NCORES = 8

# ---------------- tunables ----------------
DTYPE = "f32r"   # "f32r" (fp32 data, fp32r matmul) or "bf16"
# HW indirect DMA honors ONE offset per partition: a gather instruction
# moves exactly one 128-row chunk. GB>1 is only valid in the simulator.
GB = 1
RHS_BUFS = 6
ENT_PER_CHUNK = 128 // C   # 4 entries per 128-slot chunk


def _mdt(dtype_str):
    # float32r is bit-identical to float32 in storage; declaring the data
    # tensors as float32r end-to-end satisfies walrus's "rounded to FP32r"
    # producer check for fp32r matmuls.
    return mb.dt.bfloat16 if dtype_str == "bf16" else mb.dt.float32r


def _npdt(dtype_str):
    import ml_dtypes
    return ml_dtypes.bfloat16 if dtype_str == "bf16" else np.float32


class _Plan:
    """Host-side prep: per-core input arrays + compile-time chunk template."""

    def __init__(self, x, kidx, ridx, cidx, vals, weight, dtype_str):
        npdt = _npdt(dtype_str)
        kidx = np.asarray(kidx).astype(np.int64)
        ridx = np.asarray(ridx).astype(np.int64)
        cidx = np.asarray(cidx).astype(np.int64)
        vals = np.asarray(vals).astype(np.float32)
        weight = np.asarray(weight).astype(np.float32)
        x = np.asarray(x).astype(np.float32).reshape(C, HI, WI)

        # x2 rows: [(p, c, hi), 720] doubled parity rows
        xp = x.reshape(C, HI, WO, 2).transpose(3, 0, 1, 2)      # [2,C,HI,WO]
        x2 = np.concatenate([xp, xp], axis=-1)                  # [2,C,HI,720]
        self.x2 = np.ascontiguousarray(
            x2.reshape(2 * C * HI, 2 * WO)).astype(npdt)

        hi = cidx // WI
        wi = cidx % WI
        par = wi % 2
        s = wi // 2
        # element offset of entry's window start for channel 0
        base_off = ((par * C + 0) * HI + hi) * (2 * WO) + s     # [nnz]
        cstride = HI * 2 * WO

        counts = np.bincount(ridx, minlength=HO)
        order = np.argsort(-counts, kind="stable")              # rows by count desc
        self.nslot = math.ceil(HO / NCORES)
        # rank r -> core r % NCORES, slot r // NCORES
        self.row_of = np.full((NCORES, self.nslot), -1, dtype=np.int64)
        for r, row in enumerate(order):
            self.row_of[r % NCORES, r // NCORES] = row

        # template: chunks per slot = max entry count in that rank group
        self.nchunk = []
        for sl in range(self.nslot):
            rows = self.row_of[:, sl]
            m = max(int(counts[row]) if row >= 0 else 0 for row in rows)
            self.nchunk.append(max(1, math.ceil(m / ENT_PER_CHUNK)))
        self.totch = sum(self.nchunk)

        # entry lists per row
        ent_of_row = [[] for _ in range(HO)]
        order_e = np.argsort(ridx, kind="stable")
        sorted_r = ridx[order_e]
        bounds = np.searchsorted(sorted_r, np.arange(HO + 1))
        for row in range(HO):
            ent_of_row[row] = order_e[bounds[row]:bounds[row + 1]]

        Wt = weight.transpose(2, 1, 0)                          # [k][c][o]
        coff = (np.arange(C) * cstride).astype(np.int64)

        self.lhsT = []   # per core [128, totch*O]
        self.offT = []   # per core [128, totch] int32
        for core in range(NCORES):
            lhs_cols = []
            off_cols = []
            for sl in range(self.nslot):
                ncnk = self.nchunk[sl]
                n4 = ncnk * ENT_PER_CHUNK
                row = self.row_of[core, sl]
                ents = ent_of_row[row] if row >= 0 else np.empty(0, np.int64)
                ne = len(ents)
                v_pad = np.zeros(n4, np.float32)
                v_pad[:ne] = vals[ents]
                k_pad = np.zeros(n4, np.int64)
                k_pad[:ne] = kidx[ents]
                b_pad = np.zeros(n4, np.int64)
                b_pad[:ne] = base_off[ents]
                # lhsT stream [n4*C, O]: q = (ci*4+j)*C + c
                lw = v_pad[:, None, None] * Wt[k_pad]           # [n4, C, O]
                lhs_cols.append(
                    lw.reshape(ncnk, 128, O).transpose(1, 0, 2).reshape(
                        128, ncnk * O))
                offs = (b_pad[:, None] + coff[None, :]).reshape(ncnk, 128)
                off_cols.append(offs.T)                         # [128, ncnk]
            self.lhsT.append(np.ascontiguousarray(
                np.concatenate(lhs_cols, axis=1)).astype(npdt))
            self.offT.append(np.ascontiguousarray(
                np.concatenate(off_cols, axis=1)).astype(np.int32))


def _patch_coef(binst, coef):
    ins_l = binst.ins.ins
    dai = ins_l[0].dynamic_ap_info
    ins_l[0].dynamic_ap_info = mb.DynamicAccessPatternInfo(
        c=dai.c, actual_ap=dai.actual_ap,
        indirect_dim_max_index=dai.indirect_dim_max_index,
        offset_expr=[mb.DynamicAccessPatternOffsetExpr(
            coef=coef, aff_expr=mb.DynamicAccessPatternOffsetExprAffExpr(
                kind="IndirectArgId", arg_id=1))])


def _build_nc(plan, dtype_str):
    dt_data = _mdt(dtype_str)
    nslot, nchunk, totch = plan.nslot, plan.nchunk, plan.totch
    maxch = max(nchunk)
    nrows = 2 * C * HI

    nc = bacc.Bacc("TRN2", target_bir_lowering=False, debug=False)
    x2_d = nc.dram_tensor("x2", [nrows, 2 * WO], dt_data,
                          kind="ExternalInput").ap()
    lhsT_d = nc.dram_tensor("lhsT", [128, totch * O], dt_data,
                            kind="ExternalInput").ap()
    offT_d = nc.dram_tensor("offT", [128, totch], mb.dt.int32,
                            kind="ExternalInput").ap()
    bias_d = nc.dram_tensor("bias", [O, 1], mb.dt.float32,
                            kind="ExternalInput").ap()
    out_d = nc.dram_tensor("out", [O, nslot * WO], mb.dt.float32,
                           kind="ExternalOutput").ap()

    with tile.TileContext(nc) as tc:
        with (
            tc.tile_pool(name="const", bufs=1) as const_pool,
            tc.tile_pool(name="lhsT", bufs=2) as lhsT_pool,
            tc.tile_pool(name="rhs", bufs=RHS_BUFS) as rhs_pool,
            tc.tile_pool(name="psum", bufs=4, space="PSUM") as psum_pool,
        ):
            offT_t = const_pool.tile([128, totch], mb.dt.int32)
            nc.sync.dma_start(out=offT_t[:], in_=offT_d[:])
            bias_t = const_pool.tile([O, 1], mb.dt.float32)
            nc.sync.dma_start(out=bias_t[:], in_=bias_d[:])
            out_t = const_pool.tile([O, nslot * WO], mb.dt.float32)

            cbase = 0
            for sl in range(nslot):
                ncnk = nchunk[sl]
                lhsT_t = lhsT_pool.tile([128, maxch * O], dt_data, tag="lhsT")
                nc.sync.dma_start(
                    out=lhsT_t[:, :ncnk * O],
                    in_=lhsT_d[:, cbase * O:(cbase + ncnk) * O])
                psum_t = psum_pool.tile([O, WO], mb.dt.float32, tag="ps")
                ngrp = math.ceil(ncnk / GB)
                for g in range(ngrp):
                    gb = min(GB, ncnk - g * GB)
                    rhs_t = rhs_pool.tile([128, GB * WO], dt_data, tag="rhs")
                    binst = nc.gpsimd.indirect_dma_start(
                        out=rhs_t[:, :gb * WO],
                        out_offset=None,
                        in_=x2_d,
                        in_offset=IndirectOffsetOnAxis(
                            ap=offT_t[:, cbase + g * GB:cbase + g * GB + gb],
                            axis=0))
                    _patch_coef(binst, 1)
                    for j in range(gb):
                        ci = g * GB + j
                        nc.tensor.matmul(
                            out=psum_t[:],
                            lhsT=lhsT_t[:, ci * O:(ci + 1) * O],
                            rhs=rhs_t[:, j * WO:(j + 1) * WO],
                            start=(ci == 0),
                            stop=(ci == ncnk - 1))
                nc.vector.tensor_scalar_add(
                    out=out_t[:, sl * WO:(sl + 1) * WO],
                    in0=psum_t[:],
                    scalar1=bias_t[:])
                cbase += ncnk
            nc.sync.dma_start(out=out_d[:], in_=out_t[:])
    nc.compile()
    return nc


def kernel(x, psi_ker_idx, psi_row_idx, psi_col_idx, psi_vals, weight, bias,
           _trace=False):
    plan = _Plan(x, psi_ker_idx, psi_row_idx, psi_col_idx, psi_vals, weight,
                 DTYPE)
    nc = _build_nc(plan, DTYPE)
    bias_in = np.ascontiguousarray(
        np.asarray(bias).astype(np.float32).reshape(O, 1))
    in_maps = [
        {"x2": plan.x2, "lhsT": plan.lhsT[core], "offT": plan.offT[core],
         "bias": bias_in}
        for core in range(NCORES)
    ]
    res = bass_utils.run_bass_kernel_spmd(
        nc, in_maps, core_ids=list(range(NCORES)), trace=_trace)

    out = np.zeros((1, O, HO, WO), dtype=np.float32)
    for core in range(NCORES):
        o_core = res.results[core]["out"].reshape(O, plan.nslot, WO)
        for sl in range(plan.nslot):
            row = plan.row_of[core, sl]
            if row >= 0:
                out[0, :, row, :] = o_core[:, sl, :]
    if _trace:
        return out, res
    return out
